# Initial kernel scaffold
#
"""BertCoAttention Trainium2 kernel.

Full inputs -> shard batch across 8 NeuronCores (1 batch row each) -> full output.

Per-core dataflow (batch b):
  phase 1: load s1/s2, cast bf16, DMA-xbar transpose -> s1T/s2T [hid, seq];
           load W*, cast bf16; project:
             qT = Wq.T @ s1T   [hid_out, s1]   (+bq per-partition during evac)
             kT = Wk.T @ s2T   [hid_out, s2]   (+bk)
             v  = s2 @ Wv      [s2, hid_out]   (bv folded in at the very end)
           v_aug[:, :, h, 0:64] = v-head-slices, col 64 = ones (Z row).
  phase 2 per head h:
    scores[q,k] = qT_h.T @ kT_h scaled 1/8          (PE, K=64)
    E1 = exp(scores/8 [* exp(mask)]), Z1 = row-sums  (ACT accum_out [+DVE if mask])
    p = E1 * (1/Z1)                                  (DVE tensor_scalar, bf16 4x)
    pT = xbar-transpose(p)                           (DMA)
    E2T = exp(-pT + mask)  [skipped if cl_att=0]     (ACT, in-place)
    ctxT[65, q] = v_aug_h.T @ E2T  (row 64 = Z2)     (PE, K=128 x8)
    per q-tile: PE-transpose -> [q, 65]; out = ctx*(1/Z2) + bv  (DVE)
"""
import sys
sys.path.insert(0, "/opt/trn_rl_repo")
import numpy as np
from contextlib import ExitStack

import concourse.bass as bass
import concourse.bacc as bacc
import concourse.tile as tile
import concourse.mybir as mybir
from concourse.masks import make_identity
from concourse.bass_utils import run_bass_kernel_spmd

dt = mybir.dt
F32 = dt.float32
BF16 = dt.bfloat16
AF = mybir.ActivationFunctionType
ALU = mybir.AluOpType

S = 1024
HID = 1024
NH = 16
D = 64
PT = 8  # number of 128-row tiles in 1024
N_CORES = 8

_CACHE = {}


def _build(cl_att: bool, zero_mask: bool):
    nc = bacc.Bacc("TRN2", target_bir_lowering=False, debug=False, num_devices=N_CORES)
    s1 = nc.dram_tensor("s1", [S, HID], F32, kind="ExternalInput")
    s2 = nc.dram_tensor("s2", [S, HID], F32, kind="ExternalInput")
    msk = nc.dram_tensor("msk", [S], F32, kind="ExternalInput")
    wq = nc.dram_tensor("wq", [HID, HID], F32, kind="ExternalInput")
    wk = nc.dram_tensor("wk", [HID, HID], F32, kind="ExternalInput")
    wv = nc.dram_tensor("wv", [HID, HID], F32, kind="ExternalInput")
    bq = nc.dram_tensor("bq", [HID], F32, kind="ExternalInput")
    bk = nc.dram_tensor("bk", [HID], F32, kind="ExternalInput")
    bv = nc.dram_tensor("bv", [HID], F32, kind="ExternalInput")
    out = nc.dram_tensor("out", [S, HID], F32, kind="ExternalOutput")

    def pminor(t, n):  # [128, n] view of a flat [128*n] dram vec: [p, j] = t[j*128+p]
        return bass.AP(tensor=t, offset=0, ap=[[1, 128], [128, n]])

    def pbcast(t, n):  # [128, n] partition-broadcast of a flat [n] dram vec
        return bass.AP(tensor=t, offset=0, ap=[[0, 128], [1, n]])

    with tile.TileContext(nc) as tc, ExitStack() as ctx:
        # ---------------- persistent pools ----------------
        proj = ctx.enter_context(tc.tile_pool(name="proj", bufs=1))
        small = ctx.enter_context(tc.tile_pool(name="small", bufs=1))

        qT = proj.tile([128, PT, S], BF16)   # [hid%128, hid//128, s1]
        kT = proj.tile([128, PT, S], BF16)
        v_aug = proj.tile([128, PT, NH, D + 1], BF16)  # [s2%128, s2//128, h, d|ones]

        maskT = small.tile([128, PT], F32)
        nc.sync.dma_start(maskT[:], pminor(msk, PT))
        bqT = small.tile([128, PT], F32)
        nc.sync.dma_start(bqT[:], pminor(bq, PT))
        bkT = small.tile([128, PT], F32)
        nc.sync.dma_start(bkT[:], pminor(bk, PT))
        bvbc = small.tile([128, HID], F32)
        nc.sync.dma_start(bvbc[:], pbcast(bv, HID))
        ident = small.tile([128, 128], F32)
        make_identity(nc, ident[:])
        if not zero_mask:
            maskbc = small.tile([128, S], F32)
            nc.sync.dma_start(maskbc[:], pbcast(msk, S))
            expmaskbc = small.tile([128, S], F32)
            nc.scalar.activation(expmaskbc[:], maskbc[:], AF.Exp)

        nc.vector.memset(v_aug[:, :, :, D:D + 1], 1.0)

        # ---------------- phase 1: transpose inputs + projections ----------------
        with tc.tile_pool(name="p1stage", bufs=3) as stage_pool, \
             tc.tile_pool(name="p1bf", bufs=1) as bf_pool, \
             tc.tile_pool(name="p1w", bufs=2) as w_pool, \
             tc.tile_pool(name="p1ps", bufs=2, space="PSUM") as p1ps:

            s1T = bf_pool.tile([128, PT, S], BF16)  # [hid%128, hid//128, s]
            s2T = bf_pool.tile([128, PT, S], BF16)
            for src, dstT in ((s1, s1T), (s2, s2T)):
                natbf = bf_pool.tile([128, S], BF16, tag="natbf")
                for st in range(PT):
                    stg = stage_pool.tile([128, HID], F32, tag="stage")
                    nc.sync.dma_start(stg[:], src[st * 128:(st + 1) * 128, :])
                    natbf = bf_pool.tile([128, S], BF16, tag="natbf")
                    nc.vector.tensor_copy(natbf[:], stg[:])
                    nc.sync.dma_start(
                        dstT[:, :, st * 128:(st + 1) * 128], natbf[:], transpose=True
                    )

            # projections: qT (from s1T, wq, bq), kT (from s2T, wk, bk)
            for w_dram, bias_t, srcT, dstT2 in (
                (wq, bqT, s1T, qT),
                (wk, bkT, s2T, kT),
            ):
                wbf = w_pool.tile([128, PT, HID], BF16, tag="wbf")
                for kt in range(PT):
                    stg = stage_pool.tile([128, HID], F32, tag="stage")
                    nc.sync.dma_start(stg[:], w_dram[kt * 128:(kt + 1) * 128, :])
                    nc.vector.tensor_copy(wbf[:, kt, :], stg[:])
                for mt in range(PT):
                    ps = p1ps.tile([128, S], F32, tag="projps")
                    for kt in range(PT):
                        for nt in range(2):
                            nc.tensor.matmul(
                                ps[:, nt * 512:(nt + 1) * 512],
                                wbf[:, kt, mt * 128:(mt + 1) * 128],
                                srcT[:, kt, nt * 512:(nt + 1) * 512],
                                start=(kt == 0), stop=(kt == PT - 1),
                            )
                    nc.vector.tensor_scalar_add(
                        dstT2[:, mt, :], ps[:], bias_t[:, mt:mt + 1]
                    )

            # v projection: v[st, hid_out] = s2 @ Wv  (no bias here)
            wbf = w_pool.tile([128, PT, HID], BF16, tag="wbf")
            for kt in range(PT):
                stg = stage_pool.tile([128, HID], F32, tag="stage")
                nc.sync.dma_start(stg[:], wv[kt * 128:(kt + 1) * 128, :])
                nc.vector.tensor_copy(wbf[:, kt, :], stg[:])
            for st in range(PT):
                ps = p1ps.tile([128, S], F32, tag="projps")
                for kt in range(PT):
                    for nt in range(2):
                        nc.tensor.matmul(
                            ps[:, nt * 512:(nt + 1) * 512],
                            s2T[:, kt, st * 128:(st + 1) * 128],
                            wbf[:, kt, nt * 512:(nt + 1) * 512],
                            start=(kt == 0), stop=(kt == PT - 1),
                        )
                nc.vector.tensor_copy(
                    v_aug[:, st, :, 0:D],
                    ps[:].rearrange("p (h d) -> p h d", d=D),
                )

        # ---------------- phase 2: attention per head ----------------
        with tc.tile_pool(name="hE1", bufs=2) as e1_pool, \
             tc.tile_pool(name="hP", bufs=2) as p_pool, \
             tc.tile_pool(name="hPT", bufs=2) as pt_pool, \
             tc.tile_pool(name="hsm", bufs=2) as sm_pool, \
             tc.tile_pool(name="hout", bufs=2) as out_pool, \
             tc.tile_pool(name="scps", bufs=2, space="PSUM") as sc_ps, \
             tc.tile_pool(name="ctxps", bufs=1, space="PSUM") as ctx_ps, \
             tc.tile_pool(name="trps", bufs=2, space="PSUM") as tr_ps:

            for h in range(NH):
                mt_h = h // 2
                po = (h % 2) * 64
                E1 = e1_pool.tile([128, PT, S], BF16, tag="E1")
                Z1 = sm_pool.tile([128, PT], F32, tag="Z1")
                R1 = sm_pool.tile([128, PT], F32, tag="R1")
                P = p_pool.tile([128, PT, S], BF16, tag="P")
                PTt = pt_pool.tile([128, PT, S], BF16, tag="PT")

                for qt in range(PT):
                    ps = sc_ps.tile([128, S], F32, tag="scores")
                    for nt in range(2):
                        nc.tensor.matmul(
                            ps[:, nt * 512:(nt + 1) * 512],
                            qT[po:po + 64, mt_h, qt * 128:(qt + 1) * 128],
                            kT[po:po + 64, mt_h, nt * 512:(nt + 1) * 512],
                            start=True, stop=True,
                        )
                    if zero_mask:
                        nc.scalar.activation(
                            E1[:, qt, :], ps[:], AF.Exp,
                            scale=0.125, accum_out=Z1[:, qt:qt + 1],
                        )
                    else:
                        Eraw = sm_pool.tile([128, S], BF16, tag="Eraw")
                        nc.scalar.activation(Eraw[:], ps[:], AF.Exp, scale=0.125)
                        nc.vector.scalar_tensor_tensor(
                            out=E1[:, qt, :], in0=Eraw[:], scalar=1.0,
                            in1=expmaskbc[:],
                            op0=ALU.mult, op1=ALU.mult,
                            accum_out=Z1[:, qt:qt + 1],
                        )
                nc.vector.reciprocal(R1[:], Z1[:])
                for qt in range(PT):
                    nc.vector.tensor_scalar_mul(
                        P[:, qt, :], E1[:, qt, :], R1[:, qt:qt + 1]
                    )
                for qt in range(PT):
                    nc.sync.dma_start(
                        PTt[:, :, qt * 128:(qt + 1) * 128], P[:, qt, :], transpose=True
                    )
                if cl_att:
                    if zero_mask:
                        nc.scalar.activation(
                            PTt[:, :, :], PTt[:, :, :], AF.Exp, scale=-1.0
                        )
                    else:
                        for kt in range(PT):
                            nc.scalar.activation(
                                PTt[:, kt, :], PTt[:, kt, :], AF.Exp,
                                scale=-1.0, bias=maskT[:, kt:kt + 1],
                            )

                cps = ctx_ps.tile([D + 1, S], F32, tag="ctx")
                for kt in range(PT):
                    for nt in range(2):
                        nc.tensor.matmul(
                            cps[:, nt * 512:(nt + 1) * 512],
                            v_aug[:, kt, h, :],
                            PTt[:, kt, nt * 512:(nt + 1) * 512],
                            start=(kt == 0), stop=(kt == PT - 1),
                        )
                ctxT = out_pool.tile([D + 1, S], F32, tag="ctxT")
                nc.vector.tensor_copy(ctxT[:], cps[:])

                out_sb = out_pool.tile([128, PT, D], F32, tag="out_sb")
                for qt in range(PT):
                    trp = tr_ps.tile([128, D + 1], F32, tag="tr")
                    nc.tensor.transpose(
                        trp[:], ctxT[:, qt * 128:(qt + 1) * 128], ident[0:D + 1, 0:D + 1]
                    )
                    r2 = sm_pool.tile([128, 1], F32, tag="r2")
                    nc.vector.reciprocal(r2[:], trp[:, D:D + 1])
                    nc.vector.scalar_tensor_tensor(
                        out=out_sb[:, qt, :], in0=trp[:, 0:D], scalar=r2[:],
                        in1=bvbc[:, h * D:(h + 1) * D],
                        op0=ALU.mult, op1=ALU.add,
                    )
                nc.sync.dma_start(
                    out.rearrange("(qt p) m -> p qt m", p=128)[:, :, h * D:(h + 1) * D],
                    out_sb[:],
                )

    nc.compile()
    return nc


def _get_nc(cl_att: bool, zero_mask: bool):
    key = (cl_att, zero_mask)
    if key not in _CACHE:
        _CACHE[key] = _build(cl_att, zero_mask)
    return _CACHE[key]


def kernel(s1_hidden_states, s2_hidden_states, s2_attention_mask,
           Wq, bq, Wk, bk, Wv, bv, cl_att, _want_results=False, **_ignored):
    s1 = np.ascontiguousarray(np.asarray(s1_hidden_states, dtype=np.float32))
    s2 = np.ascontiguousarray(np.asarray(s2_hidden_states, dtype=np.float32))
    mask = np.ascontiguousarray(
        np.asarray(s2_attention_mask, dtype=np.float32).reshape(s1.shape[0], -1)
    )
    wq_ = np.ascontiguousarray(np.asarray(Wq, dtype=np.float32))
    wk_ = np.ascontiguousarray(np.asarray(Wk, dtype=np.float32))
    wv_ = np.ascontiguousarray(np.asarray(Wv, dtype=np.float32))
    bq_ = np.ascontiguousarray(np.asarray(bq, dtype=np.float32))
    bk_ = np.ascontiguousarray(np.asarray(bk, dtype=np.float32))
    bv_ = np.ascontiguousarray(np.asarray(bv, dtype=np.float32))
    cl = bool(np.asarray(cl_att))
    zero_mask = bool(np.all(mask == 0.0))

    nc = _get_nc(cl, zero_mask)
    in_maps = []
    B = s1.shape[0]
    assert B == N_CORES
    for b in range(B):
        in_maps.append({
            "s1": s1[b], "s2": s2[b], "msk": mask[b],
            "wq": wq_, "wk": wk_, "wv": wv_,
            "bq": bq_, "bk": bk_, "bv": bv_,
        })
    res = run_bass_kernel_spmd(nc, in_maps, core_ids=list(range(N_CORES)))
    out = np.stack([res.results[b]["out"] for b in range(B)], axis=0)
    if _want_results:
        return out, res
    return out


# revision 4
# speedup vs baseline: 4.6273x; 4.6273x over previous
"""BertCoAttention Trainium2 kernel.

Full inputs -> shard batch across 8 NeuronCores (1 batch row each) -> full output.

Per-core dataflow (batch b):
  phase 1: load s1/s2, cast bf16, DMA-xbar transpose -> s1T/s2T [hid, seq];
           load W*, cast bf16; project:
             qT = Wq.T @ s1T   [hid_out, s1]   (+bq per-partition during evac)
             kT = Wk.T @ s2T   [hid_out, s2]   (+bk)
             v  = s2 @ Wv      [s2, hid_out]   (bv folded in at the very end)
           v_aug[:, :, h, 0:64] = v-head-slices, col 64 = ones (Z row).
  phase 2 per head h:
    scores[q,k] = qT_h.T @ kT_h scaled 1/8          (PE, K=64)
    E1 = exp(scores/8 [* exp(mask)]), Z1 = row-sums  (ACT accum_out [+DVE if mask])
    p = E1 * (1/Z1)                                  (DVE tensor_scalar, bf16 4x)
    pT = xbar-transpose(p)                           (DMA)
    E2T = exp(-pT + mask)  [skipped if cl_att=0]     (ACT, in-place)
    ctxT[65, q] = v_aug_h.T @ E2T  (row 64 = Z2)     (PE, K=128 x8)
    per q-tile: PE-transpose -> [q, 65]; out = ctx*(1/Z2) + bv  (DVE)
"""
import sys
sys.path.insert(0, "/opt/trn_rl_repo")
import numpy as np
from contextlib import ExitStack

import concourse.bass as bass
import concourse.bacc as bacc
import concourse.tile as tile
import concourse.mybir as mybir
from concourse.masks import make_identity
from concourse.bass_utils import run_bass_kernel_spmd

dt = mybir.dt
F32 = dt.float32
BF16 = dt.bfloat16
AF = mybir.ActivationFunctionType
ALU = mybir.AluOpType

S = 1024
HID = 1024
NH = 16
D = 64
PT = 8  # number of 128-row tiles in 1024
N_CORES = 8

_CACHE = {}


def _build(cl_att: bool, zero_mask: bool, repeat: int = 1):
    nc = bacc.Bacc("TRN2", target_bir_lowering=False, debug=False, num_devices=N_CORES)
    s1 = nc.dram_tensor("s1", [S, HID], F32, kind="ExternalInput")
    s2 = nc.dram_tensor("s2", [S, HID], F32, kind="ExternalInput")
    msk = nc.dram_tensor("msk", [S], F32, kind="ExternalInput")
    wq = nc.dram_tensor("wq", [HID, HID], F32, kind="ExternalInput")
    wk = nc.dram_tensor("wk", [HID, HID], F32, kind="ExternalInput")
    wv = nc.dram_tensor("wv", [HID, HID], F32, kind="ExternalInput")
    bq = nc.dram_tensor("bq", [HID], F32, kind="ExternalInput")
    bk = nc.dram_tensor("bk", [HID], F32, kind="ExternalInput")
    bv = nc.dram_tensor("bv", [HID], F32, kind="ExternalInput")
    out = nc.dram_tensor("out", [S, HID], F32, kind="ExternalOutput")

    def pminor(t, n):  # [128, n] view of a flat [128*n] dram vec: [p, j] = t[j*128+p]
        return bass.AP(tensor=t, offset=0, ap=[[1, 128], [128, n]])

    def pbcast(t, n):  # [128, n] partition-broadcast of a flat [n] dram vec
        return bass.AP(tensor=t, offset=0, ap=[[0, 128], [1, n]])

    with tile.TileContext(nc) as tc:
      for _rep in range(repeat):
       with ExitStack() as ctx:
        # ---------------- persistent pools ----------------
        proj = ctx.enter_context(tc.tile_pool(name="proj", bufs=1))
        small = ctx.enter_context(tc.tile_pool(name="small", bufs=1))

        qT = proj.tile([128, PT, S], BF16)   # [hid%128, hid//128, s1]
        kT = proj.tile([128, PT, S], BF16)
        v_aug = proj.tile([128, PT, NH, D + 1], BF16)  # [s2%128, s2//128, h, d|ones]

        maskT = small.tile([128, PT], F32)
        nc.sync.dma_start(maskT[:], pminor(msk, PT))
        bqT = small.tile([128, PT], F32)
        nc.sync.dma_start(bqT[:], pminor(bq, PT))
        bkT = small.tile([128, PT], F32)
        nc.sync.dma_start(bkT[:], pminor(bk, PT))
        bvbc = small.tile([128, HID], F32)
        nc.sync.dma_start(bvbc[:], pbcast(bv, HID))
        ident = small.tile([128, 128], F32)
        make_identity(nc, ident[:])
        if not zero_mask:
            maskbc = small.tile([128, S], F32)
            nc.sync.dma_start(maskbc[:], pbcast(msk, S))
            expmaskbc = small.tile([128, S], F32)
            nc.scalar.activation(expmaskbc[:], maskbc[:], AF.Exp)

        nc.vector.memset(v_aug[:, :, :, D:D + 1], 1.0)

        # ---------------- phase 1: transpose inputs + projections ----------------
        with tc.tile_pool(name="p1stage", bufs=3) as stage_pool, \
             tc.tile_pool(name="p1bf", bufs=1) as bf_pool, \
             tc.tile_pool(name="p1w", bufs=2) as w_pool, \
             tc.tile_pool(name="p1ps", bufs=2, space="PSUM") as p1ps:

            s1T = bf_pool.tile([128, PT, S], BF16)  # [hid%128, hid//128, s]
            s2T = bf_pool.tile([128, PT, S], BF16)
            for src, dstT in ((s1, s1T), (s2, s2T)):
                natbf = bf_pool.tile([128, S], BF16, tag="natbf")
                for st in range(PT):
                    stg = stage_pool.tile([128, HID], F32, tag="stage")
                    nc.sync.dma_start(stg[:], src[st * 128:(st + 1) * 128, :])
                    natbf = bf_pool.tile([128, S], BF16, tag="natbf")
                    nc.vector.tensor_copy(natbf[:], stg[:])
                    nc.sync.dma_start(
                        dstT[:, :, st * 128:(st + 1) * 128], natbf[:], transpose=True
                    )

            # projections: qT (from s1T, wq, bq), kT (from s2T, wk, bk)
            for w_dram, bias_t, srcT, dstT2 in (
                (wq, bqT, s1T, qT),
                (wk, bkT, s2T, kT),
            ):
                wbf = w_pool.tile([128, PT, HID], BF16, tag="wbf")
                for kt in range(PT):
                    stg = stage_pool.tile([128, HID], F32, tag="stage")
                    nc.sync.dma_start(stg[:], w_dram[kt * 128:(kt + 1) * 128, :])
                    nc.vector.tensor_copy(wbf[:, kt, :], stg[:])
                for mt in range(PT):
                    ps = p1ps.tile([128, S], F32, tag="projps")
                    for kt in range(PT):
                        for nt in range(2):
                            nc.tensor.matmul(
                                ps[:, nt * 512:(nt + 1) * 512],
                                wbf[:, kt, mt * 128:(mt + 1) * 128],
                                srcT[:, kt, nt * 512:(nt + 1) * 512],
                                start=(kt == 0), stop=(kt == PT - 1),
                            )
                    nc.vector.tensor_scalar_add(
                        dstT2[:, mt, :], ps[:], bias_t[:, mt:mt + 1]
                    )

            # v projection: v[st, hid_out] = s2 @ Wv  (no bias here)
            wbf = w_pool.tile([128, PT, HID], BF16, tag="wbf")
            for kt in range(PT):
                stg = stage_pool.tile([128, HID], F32, tag="stage")
                nc.sync.dma_start(stg[:], wv[kt * 128:(kt + 1) * 128, :])
                nc.vector.tensor_copy(wbf[:, kt, :], stg[:])
            for st in range(PT):
                ps = p1ps.tile([128, S], F32, tag="projps")
                for kt in range(PT):
                    for nt in range(2):
                        nc.tensor.matmul(
                            ps[:, nt * 512:(nt + 1) * 512],
                            s2T[:, kt, st * 128:(st + 1) * 128],
                            wbf[:, kt, nt * 512:(nt + 1) * 512],
                            start=(kt == 0), stop=(kt == PT - 1),
                        )
                nc.vector.tensor_copy(
                    v_aug[:, st, :, 0:D],
                    ps[:].rearrange("p (h d) -> p h d", d=D),
                )

        # ---------------- phase 2: attention per head ----------------
        with tc.tile_pool(name="hE1", bufs=2) as e1_pool, \
             tc.tile_pool(name="hP", bufs=2) as p_pool, \
             tc.tile_pool(name="hPT", bufs=2) as pt_pool, \
             tc.tile_pool(name="hsm", bufs=2) as sm_pool, \
             tc.tile_pool(name="hout", bufs=2) as out_pool, \
             tc.tile_pool(name="scps", bufs=2, space="PSUM") as sc_ps, \
             tc.tile_pool(name="ctxps", bufs=1, space="PSUM") as ctx_ps, \
             tc.tile_pool(name="trps", bufs=2, space="PSUM") as tr_ps:

            for h in range(NH):
                mt_h = h // 2
                po = (h % 2) * 64
                E1 = e1_pool.tile([128, PT, S], BF16, tag="E1")
                Z1 = sm_pool.tile([128, PT], F32, tag="Z1")
                R1 = sm_pool.tile([128, PT], F32, tag="R1")
                P = p_pool.tile([128, PT, S], BF16, tag="P")
                PTt = pt_pool.tile([128, PT, S], BF16, tag="PT")

                for qt in range(PT):
                    ps = sc_ps.tile([128, S], F32, tag="scores")
                    for nt in range(2):
                        nc.tensor.matmul(
                            ps[:, nt * 512:(nt + 1) * 512],
                            qT[po:po + 64, mt_h, qt * 128:(qt + 1) * 128],
                            kT[po:po + 64, mt_h, nt * 512:(nt + 1) * 512],
                            start=True, stop=True,
                        )
                    if zero_mask:
                        nc.scalar.activation(
                            E1[:, qt, :], ps[:], AF.Exp,
                            scale=0.125, accum_out=Z1[:, qt:qt + 1],
                        )
                    else:
                        Eraw = sm_pool.tile([128, S], BF16, tag="Eraw")
                        nc.scalar.activation(Eraw[:], ps[:], AF.Exp, scale=0.125)
                        nc.vector.scalar_tensor_tensor(
                            out=E1[:, qt, :], in0=Eraw[:], scalar=1.0,
                            in1=expmaskbc[:],
                            op0=ALU.mult, op1=ALU.mult,
                            accum_out=Z1[:, qt:qt + 1],
                        )
                nc.vector.reciprocal(R1[:], Z1[:])
                for qt in range(PT):
                    nc.vector.tensor_scalar_mul(
                        P[:, qt, :], E1[:, qt, :], R1[:, qt:qt + 1]
                    )
                for qt in range(PT):
                    nc.sync.dma_start(
                        PTt[:, :, qt * 128:(qt + 1) * 128], P[:, qt, :], transpose=True
                    )
                if cl_att:
                    if zero_mask:
                        nc.scalar.activation(
                            PTt[:, :, :], PTt[:, :, :], AF.Exp, scale=-1.0
                        )
                    else:
                        for kt in range(PT):
                            nc.scalar.activation(
                                PTt[:, kt, :], PTt[:, kt, :], AF.Exp,
                                scale=-1.0, bias=maskT[:, kt:kt + 1],
                            )

                cps = ctx_ps.tile([D + 1, S], F32, tag="ctx")
                for kt in range(PT):
                    for nt in range(2):
                        nc.tensor.matmul(
                            cps[:, nt * 512:(nt + 1) * 512],
                            v_aug[:, kt, h, :],
                            PTt[:, kt, nt * 512:(nt + 1) * 512],
                            start=(kt == 0), stop=(kt == PT - 1),
                        )
                ctxT = out_pool.tile([D + 1, S], F32, tag="ctxT")
                nc.vector.tensor_copy(ctxT[:], cps[:])

                out_sb = out_pool.tile([128, PT, D], F32, tag="out_sb")
                for qt in range(PT):
                    trp = tr_ps.tile([128, D + 1], F32, tag="tr")
                    nc.tensor.transpose(
                        trp[:], ctxT[:, qt * 128:(qt + 1) * 128], ident[0:D + 1, 0:D + 1]
                    )
                    r2 = sm_pool.tile([128, 1], F32, tag="r2")
                    nc.vector.reciprocal(r2[:], trp[:, D:D + 1])
                    nc.vector.scalar_tensor_tensor(
                        out=out_sb[:, qt, :], in0=trp[:, 0:D], scalar=r2[:],
                        in1=bvbc[:, h * D:(h + 1) * D],
                        op0=ALU.mult, op1=ALU.add,
                    )
                nc.sync.dma_start(
                    out.rearrange("(qt p) m -> p qt m", p=128)[:, :, h * D:(h + 1) * D],
                    out_sb[:],
                )

    nc.compile()
    return nc


def _get_nc(cl_att: bool, zero_mask: bool, repeat: int = 1):
    key = (cl_att, zero_mask, repeat)
    if key not in _CACHE:
        _CACHE[key] = _build(cl_att, zero_mask, repeat)
    return _CACHE[key]


def kernel(s1_hidden_states, s2_hidden_states, s2_attention_mask,
           Wq, bq, Wk, bk, Wv, bv, cl_att, _want_results=False, **_ignored):
    s1 = np.ascontiguousarray(np.asarray(s1_hidden_states, dtype=np.float32))
    s2 = np.ascontiguousarray(np.asarray(s2_hidden_states, dtype=np.float32))
    mask = np.ascontiguousarray(
        np.asarray(s2_attention_mask, dtype=np.float32).reshape(s1.shape[0], -1)
    )
    wq_ = np.ascontiguousarray(np.asarray(Wq, dtype=np.float32))
    wk_ = np.ascontiguousarray(np.asarray(Wk, dtype=np.float32))
    wv_ = np.ascontiguousarray(np.asarray(Wv, dtype=np.float32))
    bq_ = np.ascontiguousarray(np.asarray(bq, dtype=np.float32))
    bk_ = np.ascontiguousarray(np.asarray(bk, dtype=np.float32))
    bv_ = np.ascontiguousarray(np.asarray(bv, dtype=np.float32))
    cl = bool(np.asarray(cl_att))
    zero_mask = bool(np.all(mask == 0.0))

    nc = _get_nc(cl, zero_mask)
    in_maps = []
    B = s1.shape[0]
    assert B == N_CORES
    for b in range(B):
        in_maps.append({
            "s1": s1[b], "s2": s2[b], "msk": mask[b],
            "wq": wq_, "wk": wk_, "wv": wv_,
            "bq": bq_, "bk": bk_, "bv": bv_,
        })
    res = run_bass_kernel_spmd(nc, in_maps, core_ids=list(range(N_CORES)))
    out = np.stack([res.results[b]["out"] for b in range(B)], axis=0)
    if _want_results:
        return out, res
    return out
